# revision 75
# baseline (speedup 1.0000x reference)
"""LIF neuron kernel for Trainium2, 8-core SPMD (batch-sharded), bit-packed output.

Reference semantics per timestep t (fp32, TAU=0.5):
    u   = 0.5*m + x_t          # leaky integrate
    s   = (u >= thresh)        # fire (output, 1.0/0.0)
    m'  = u * (u < thresh)     # hard reset

Design ("marker fusion") -- one DVE uop per timestep total:
  * Host pre-scales x~ = x / thresh, so the threshold becomes the constant
    1.0 and no thresh tile is needed on-device.  (fp32 rescale perturbs
    spike decisions only within ~1ulp of threshold; empirically 0 flips.)
  * Single fused custom DVE op per step carries BOTH membrane and spike in
    one value:
        m    = y * (y < 1)                 # decode: marker (>=2^64) -> reset
        v    = m + x~_t
        y'   = v < 1 ? v*0.5 : 2^(64+j)    # membrane, or huge spike marker
    where j = t mod 16 is the bit index within the packing group.
  * PE accumulates y directly (identity stationary, fp32r) into PSUM fp32:
    markers are exact powers of two; membrane residues (|.|<2^10) vanish
    below ulp(2^64)=2^41 in the fp32 accumulate, so after a 16-step group
    PSUM = (sum_j s_j 2^j) * 2^64 EXACTLY (or tiny junk if no spikes).
  * ACT evicts PSUM with scale=2^-64 into uint16: exact packed spikes.
    Output DMA is 16x smaller than fp32 spikes (0.46 MB vs 13.1 MB/core).
  * No Pool/GpSimd work at all; DMA-in (13.1 MB/core fp32) is the roofline.

Per-core layout: batches 8c..8c+7.  Lanes (b_local, n) map to SBUF as
partition p = b_local*16 + (n // 256), free f = n % 256.  x is host-
transposed to [P, T, F] (partition-major) so every DMA is contiguous
per partition (>=2KB descriptors, full bus bandwidth).
"""

import os

import numpy as np

# reset cores at NRT init: recovers cleanly if a previous process left the
# device wedged (must be set before the neuron runtime initializes)
os.environ.setdefault("NEURON_RT_RESET_CORES", "1")

import concourse.bass as bass
import concourse.bacc as bacc
import concourse.mybir as mybir
from concourse import tile
from concourse.bass_utils import run_bass_kernel_spmd

B, T, N = 64, 100, 4096
NCORES = 8
BL = B // NCORES          # local batches per core
C = 16                    # feature chunks -> partitions
F = N // C                # 256 features per chunk
P = BL * C                # 128 partitions
GL = 16                   # timesteps packed per uint16 output group
NG = (T + GL - 1) // GL   # 7 groups (6x16 + 1x4)
MARK = 2.0 ** 64          # spike marker base (marker = MARK * 2^j)

# Input DMA chunk sizes: small first chunks shrink pipeline-fill skew,
# small last chunks shrink the drain tail.
# Stall-free chunk schedule: DVE consumes a step every ~388 ns while DMA
# supplies one every ~364 ns, so supply builds a 24 ns/step lead.  Constant
# 2-step chunks while the lead ramps, then grow once the accumulated lead
# covers each size jump (364*(s+1) - 388*s <= lead).
# zero-stall condition (supply-bound ramp): 364*c_k + 1006 + first_transfer
# <= dve_start + 388*c_{k-1}  =>  s_k <= 2 + 0.066*c_{k-1}
IN_CHUNKS = [2]*7 + [3]*5 + [4]*16 + [2, 2, 1, 1, 1]
assert sum(IN_CHUNKS) == T
HF = F // 2               # half-row column split (two independent DVE chains)

_F32 = mybir.dt.float32
_F32R = mybir.dt.float32r
_U16 = mybir.dt.uint16
_ALU = mybir.AluOpType

# ---------------------------------------------------------------- custom op --

_LIF_OP = None


def _register_lif_op():
    """Fused LIF step with spike marker:
    y' = select(y*(y<1) + x < 1, (y*(y<1) + x) * s0, s1), one uop."""
    global _LIF_OP
    if _LIF_OP is not None:
        return _LIF_OP
    from concourse.dve_spec import C0, C1, Spec, Src0, Src1, One, select, lower
    from concourse.dve_uop import DveOpSpec
    from concourse import dve_ops as dom

    name = "LIF_MARK_ANT"
    for op in dom.OPS:
        if op.name == name:
            _LIF_OP = op
            return op

    v = Src0 * (Src0 < One) + Src1
    spec = Spec(
        body=select(v < One, v * C0, C1),
        reference=lambda in0, in1, s0, s1, imm2: np.where(
            (in0 * (in0 < np.float32(1.0)) + in1) < np.float32(1.0),
            ((in0 * (in0 < np.float32(1.0)) + in1) * np.float32(s0)),
            np.float32(s1),
        ).astype(np.float32),
    )
    shas = {}
    for ver in ("v3", "v4"):
        try:
            tmp = DveOpSpec(name=name, opcode=None, uops=lower(spec, ver=ver), rd1_en=True)
            shas[ver] = tmp.sha(ver)
        except Exception:
            pass
    op = dom.DveOp(name, spec, subdim=False, uops_sha=shas)
    dom.OPS.append(op)
    dom._SUB_OPCODE_FOR_NAME[name] = dom._CUSTOM_DVE_ROW_BASE + len(dom.OPS) - 1
    dom.CUSTOM_DVE_SPECS[name] = spec
    _LIF_OP = op
    return op


# ------------------------------------------------------------------ program --

_NC_CACHE = {}


def _build_bass():
    if "nc" in _NC_CACHE:
        return _NC_CACHE["nc"]
    lif_op = _register_lif_op()

    nc = bacc.Bacc("TRN2", name="lif_pack")
    xt = nc.dram_tensor("xt", [P, T, F], _F32, kind="ExternalInput")
    pk = nc.dram_tensor("pk", [P, NG, F], _U16, kind="ExternalOutput")

    chunk_start = {}
    t0 = 0
    for L in IN_CHUNKS:
        chunk_start[t0] = L
        t0 += L

    with tile.TileContext(nc) as tc:
        with (
            tc.tile_pool(name="const", bufs=1) as cpool,
            tc.tile_pool(name="xin", bufs=8) as xpool,
            tc.tile_pool(name="ybuf", bufs=3) as ypool,
            tc.tile_pool(name="outp", bufs=3) as opool,
            tc.tile_pool(name="ps", bufs=2, space="PSUM") as ppool,
        ):
            y_init = cpool.tile([P, F], _F32)
            nc.vector.memset(y_init[:], 0.0)

            # identity built on-device (idle Pool + one early DVE copy):
            # iota(p,f) = p - f, is_equal 0 -> {1.0, 0.0}, then a DVE copy
            # retypes to fp32r for the matmul's producer-dtype check
            idio = cpool.tile([P, P], mybir.dt.int32)
            nc.gpsimd.iota(idio[:], [[-1, P]], base=0, channel_multiplier=1)
            idf = cpool.tile([P, P], _F32)
            nc.gpsimd.tensor_scalar(
                out=idf[:], in0=idio[:], scalar1=0.0, scalar2=None,
                op0=_ALU.is_equal,
            )
            id32 = cpool.tile([P, P], _F32R)
            nc.vector.tensor_copy(id32[:], idf[:])

            def emit_pack(ps, yg, glen, g):
                """matmuls + evict + out-DMA for a finished group."""
                for j in range(glen):
                    nc.tensor.matmul(
                        ps[:], id32[:], yg[:, j, :].bitcast(_F32R),
                        start=(j == 0), stop=(j == glen - 1),
                    )
                ot = opool.tile([P, F], _U16, name=f"ot{g}")
                nc.scalar.activation(
                    ot[:], ps[:], mybir.ActivationFunctionType.Copy,
                    scale=float(2.0 ** -64),
                )
                # last two groups: SP queue is idle by then, and keeping them
                # off ACT's sequencer lets evict(g+1) decode immediately
                eng = nc.sync if g >= NG - 2 else nc.scalar
                eng.dma_start(pk[:, g, :], ot[:])

            x_cur = None
            cur_t0 = 0
            yg_prev = None
            prev_glen = 0
            pending = None        # (ps, yg, glen, g) awaiting pack emission
            for g in range(NG):
                glen = min(GL, T - g * GL)
                ps = ppool.tile([P, F], _F32)
                yg = ypool.tile([P, glen, F], _F32)
                for j in range(glen):
                    t = g * GL + j
                    if t in chunk_start:
                        L = chunk_start[t]
                        x_cur = xpool.tile([P, L, F], _F32)
                        nc.sync.dma_start(x_cur[:], xt[:, t:t + L, :])
                        cur_t0 = t

                    # two independent half-row chains: the second op's engine
                    # time hides the first's write->read semaphore latency
                    for h in range(2):
                        cs = slice(h * HF, (h + 1) * HF)
                        if j > 0:
                            y_old = yg[:, j - 1, cs]
                        elif g == 0:
                            y_old = y_init[:, cs]
                        else:
                            y_old = yg_prev[:, prev_glen - 1, cs]
                        x_in = x_cur[:, t - cur_t0, cs]
                        # out is typed fp32r (same 4-byte storage) so the BIR
                        # verifier accepts it as the fp32r matmul's producer;
                        # the recurrence reads it back through the fp32 view
                        nc.vector._custom_dve(
                            lif_op, out=yg[:, j, cs].bitcast(_F32R), in0=y_old,
                            in1=x_in,
                            s0=0.5, s1=float(MARK * (1 << j)),
                        )
                yg_prev = yg
                prev_glen = glen
                if pending is not None:
                    emit_pack(*pending)
                pending = (ps, yg, glen, g)
            emit_pack(*pending)

    nc.finalize()
    _NC_CACHE["nc"] = nc
    return nc


# -------------------------------------------------------------------- entry --

def _run(x, thresh, trace=False):
    nc = _build_bass()
    x = np.ascontiguousarray(x, dtype=np.float32)
    thresh = np.ascontiguousarray(thresh, dtype=np.float32)
    xs = (x / thresh).astype(np.float32)                  # [B, T, N]
    in_maps = []
    for c in range(NCORES):
        xc = (
            xs[c * BL:(c + 1) * BL]
            .reshape(BL, T, C, F)
            .transpose(0, 2, 1, 3)                        # [BL, C, T, F]
            .reshape(P, T, F)
        )
        in_maps.append({"xt": np.ascontiguousarray(xc)})

    res = run_bass_kernel_spmd(
        nc, in_maps, core_ids=list(range(NCORES)), trace=trace
    )
    outs = []
    for c in range(NCORES):
        pkc = np.asarray(res.results[c]["pk"])            # [P, NG, F] uint16
        bits = np.unpackbits(
            pkc.view(np.uint8).reshape(P, NG, F, 2), axis=-1, bitorder="little"
        )                                                 # [P, NG, F, 16]
        a = (
            bits.reshape(BL, C, NG, F, GL)
            .transpose(0, 2, 4, 1, 3)                     # [BL, NG, GL, C, F]
            .reshape(BL, NG * GL, N)[:, :T, :]
        )
        outs.append(a.astype(np.float32))
    return np.concatenate(outs, axis=0), res


def kernel(x, thresh):
    out, _ = _run(x, thresh, trace=False)
    return out
